# revision 13
# baseline (speedup 1.0000x reference)
"""Block-diagonal linear (DiagonalLinear) Trainium2 kernel.

y[:, n*256:(n+1)*256] = x[:, n*256:(n+1)*256] @ W[n].T + b[n]  for n in 0..63

Sharding: expert-parallel over the 64 blocks — core c owns blocks
[8c, 8c+8). Host pre-transposes x and W so the contraction dim (ip) lands
on SBUF partitions; device computes yT[n, op, batch] per block with
float32r matmuls (full-rate fp32); host transposes the result back.
"""

from contextlib import ExitStack

import numpy as np

import concourse.bacc as bacc
import concourse.bass as bass
import concourse.tile as tile
from concourse import mybir
from concourse.bass_utils import run_bass_kernel_spmd

N_COPIES, IP, OP, BATCH = 64, 256, 256, 4096
N_CORES = 8
BPC = N_COPIES // N_CORES  # blocks per core
P = 128
KC = IP // P  # contraction chunks per block
MC = OP // P  # output-partition chunks per block
FREE = 512  # moving free dim per matmul (one PSUM bank of fp32)
JN = BATCH // FREE

_prog_cache = {}


def _build_program():
    nc = bacc.Bacc("TRN2", target_bir_lowering=False, debug=False)
    f32 = mybir.dt.float32
    f16 = mybir.dt.float16

    # x, W and y move as fp16: halves both input and output DMA traffic;
    # matmul accumulates in fp32 PSUM, bias-add in fp32, y rounded to fp16
    # on store (y error ~3.5e-4 scale-rel vs the fp32 reference).
    xt = nc.dram_tensor("xt", [BPC, IP, BATCH], f16, kind="ExternalInput").ap()
    wt = nc.dram_tensor("wt", [BPC, IP, OP], f16, kind="ExternalInput").ap()
    bb = nc.dram_tensor("bb", [BPC, OP], f32, kind="ExternalInput").ap()
    yt = nc.dram_tensor("yt", [BPC, OP, BATCH], f16, kind="ExternalOutput").ap()

    with tile.TileContext(nc) as tc, ExitStack() as ctx:
        const = ctx.enter_context(tc.tile_pool(name="const", bufs=1))
        xpool = ctx.enter_context(tc.tile_pool(name="x", bufs=4))
        ypool = ctx.enter_context(tc.tile_pool(name="y", bufs=6))
        psum = ctx.enter_context(tc.tile_pool(name="ps", bufs=8, space="PSUM"))

        # All weights (2 MiB) and biases for this core's 8 blocks, loaded once.
        # Input loads go on nc.sync's HWDGE ring; output stores on nc.scalar's
        # ring, so stores never head-of-line-block the next input load.
        wtile = const.tile([P, BPC * KC, OP], f16)
        nc.scalar.dma_start(
            out=wtile[:], in_=wt.rearrange("n (kc p) o -> p (n kc) o", p=P)
        )
        btile = const.tile([P, BPC * MC], f32)
        nc.scalar.dma_start(
            out=btile[:], in_=bb.rearrange("n (m p) -> p (n m)", p=P)
        )

        for n in range(BPC):
            xtile = xpool.tile([P, KC, BATCH], f16)
            nc.sync.dma_start(
                out=xtile[:], in_=xt[n].rearrange("(kc p) f -> p kc f", p=P)
            )
            for m in range(MC):
                ytile = ypool.tile([P, BATCH], f16)
                pss = [psum.tile([P, FREE], f32, name="psj") for _ in range(JN)]
                # kc outer: the stationary weight chunk stays loaded across
                # all 8 batch chunks (1 LDWEIGHTS per 8 matmuls).
                for kc in range(KC):
                    for j in range(JN):
                        nc.tensor.matmul(
                            pss[j][:],
                            wtile[:, n * KC + kc, bass.ts(m, P)],
                            xtile[:, kc, bass.ts(j, FREE)],
                            start=(kc == 0),
                            stop=(kc == KC - 1),
                        )
                bias = btile[:, n * MC + m : n * MC + m + 1]
                for j in range(JN):
                    # split PSUM evictions across DVE and ACT
                    if j % 2 == 0:
                        nc.vector.tensor_scalar_add(
                            ytile[:, bass.ts(j, FREE)], pss[j][:], bias
                        )
                    else:
                        nc.scalar.activation(
                            ytile[:, bass.ts(j, FREE)],
                            pss[j][:],
                            mybir.ActivationFunctionType.Identity,
                            bias=bias,
                        )
                nc.scalar.dma_start(out=yt[n, bass.ts(m, P)], in_=ytile[:])

    nc.compile()
    return nc


def _get_program():
    if "nc" not in _prog_cache:
        _prog_cache["nc"] = _build_program()
    return _prog_cache["nc"]


def _run(x, W, b, **spmd_kwargs):
    x = np.ascontiguousarray(x, dtype=np.float32)
    W = np.ascontiguousarray(W, dtype=np.float32)
    b = np.ascontiguousarray(b, dtype=np.float32)

    # [B, n*ip] -> [n, ip, B]; two-step transpose is much faster than a
    # direct (1, 2, 0) permute copy (cache-friendly inner strides).
    xa = x.reshape(BATCH, N_COPIES, IP).transpose(1, 0, 2).astype(np.float16)
    xT = np.ascontiguousarray(xa.transpose(0, 2, 1))  # [n, ip, B] fp16
    wT = np.ascontiguousarray(W.transpose(0, 2, 1).astype(np.float16))  # [n, ip, op]

    nc = _get_program()
    in_maps = [
        {
            "xt": xT[c * BPC : (c + 1) * BPC],
            "wt": wT[c * BPC : (c + 1) * BPC],
            "bb": b[c * BPC : (c + 1) * BPC],
        }
        for c in range(N_CORES)
    ]
    res = run_bass_kernel_spmd(nc, in_maps, core_ids=list(range(N_CORES)), **spmd_kwargs)

    yT = np.concatenate([res.results[c]["yt"] for c in range(N_CORES)], axis=0).astype(np.float32)
    # [n, op, B] -> [B, n, op] -> [B, n*op]
    ya = np.ascontiguousarray(yT.transpose(0, 2, 1))  # [n, B, op]
    y = np.ascontiguousarray(ya.transpose(1, 0, 2)).reshape(BATCH, N_COPIES * OP)
    return y, res


def kernel(x, W, b):
    y, _ = _run(x, W, b)
    return y


# revision 14
# speedup vs baseline: 1.1090x; 1.1090x over previous
"""Block-diagonal linear (DiagonalLinear) Trainium2 kernel.

y[:, n*256:(n+1)*256] = x[:, n*256:(n+1)*256] @ W[n].T + b[n]  for n in 0..63

Sharding: expert-parallel over the 64 blocks — core c owns blocks
[8c, 8c+8). Host pre-transposes x and W so the contraction dim (ip) lands
on SBUF partitions; device computes yT[n, op, batch] per block with
float32r matmuls (full-rate fp32); host transposes the result back.
"""

from contextlib import ExitStack

import numpy as np

import concourse.bacc as bacc
import concourse.bass as bass
import concourse.tile as tile
from concourse import mybir
from concourse.bass_utils import run_bass_kernel_spmd

N_COPIES, IP, OP, BATCH = 64, 256, 256, 4096
N_CORES = 8
BPC = N_COPIES // N_CORES  # blocks per core
P = 128
KC = IP // P  # contraction chunks per block
MC = OP // P  # output-partition chunks per block
FREE = 512  # moving free dim per matmul (one PSUM bank of fp32)
JN = BATCH // FREE

_prog_cache = {}


def _build_program():
    nc = bacc.Bacc("TRN2", target_bir_lowering=False, debug=False)
    f32 = mybir.dt.float32
    f16 = mybir.dt.float16

    # x, W and y move as fp16: halves both input and output DMA traffic;
    # matmul accumulates in fp32 PSUM, bias-add in fp32, y rounded to fp16
    # on store (y error ~3.5e-4 scale-rel vs the fp32 reference).
    xt = nc.dram_tensor("xt", [BPC, IP, BATCH], f16, kind="ExternalInput").ap()
    wt = nc.dram_tensor("wt", [BPC, IP, OP], f16, kind="ExternalInput").ap()
    bb = nc.dram_tensor("bb", [BPC, OP], f32, kind="ExternalInput").ap()
    yt = nc.dram_tensor("yt", [BPC, OP, BATCH], f16, kind="ExternalOutput").ap()

    with tile.TileContext(nc) as tc, ExitStack() as ctx:
        const = ctx.enter_context(tc.tile_pool(name="const", bufs=1))
        xpool = ctx.enter_context(tc.tile_pool(name="x", bufs=4))
        ypool = ctx.enter_context(tc.tile_pool(name="y", bufs=6))
        psum = ctx.enter_context(tc.tile_pool(name="ps", bufs=8, space="PSUM"))

        # All weights (2 MiB) and biases for this core's 8 blocks, loaded once.
        # Input loads go on nc.sync's HWDGE ring; output stores on nc.scalar's
        # ring, so stores never head-of-line-block the next input load.
        wtile = const.tile([P, BPC * KC, OP], f16)
        nc.scalar.dma_start(
            out=wtile[:], in_=wt.rearrange("n (kc p) o -> p (n kc) o", p=P)
        )
        btile = const.tile([P, BPC * MC], f32)
        nc.scalar.dma_start(
            out=btile[:], in_=bb.rearrange("n (m p) -> p (n m)", p=P)
        )

        for n in range(BPC):
            xtile = xpool.tile([P, KC, BATCH], f16)
            for kc in range(KC):
                nc.sync.dma_start(out=xtile[:, kc], in_=xt[n, bass.ts(kc, P)])
            for m in range(MC):
                ytile = ypool.tile([P, BATCH], f16)
                pss = [psum.tile([P, FREE], f32, name="psj") for _ in range(JN)]
                # kc outer: the stationary weight chunk stays loaded across
                # all 8 batch chunks (1 LDWEIGHTS per 8 matmuls).
                for kc in range(KC):
                    for j in range(JN):
                        nc.tensor.matmul(
                            pss[j][:],
                            wtile[:, n * KC + kc, bass.ts(m, P)],
                            xtile[:, kc, bass.ts(j, FREE)],
                            start=(kc == 0),
                            stop=(kc == KC - 1),
                        )
                bias = btile[:, n * MC + m : n * MC + m + 1]
                for j in range(JN):
                    # split PSUM evictions across DVE and ACT
                    if j % 2 == 0:
                        nc.vector.tensor_scalar_add(
                            ytile[:, bass.ts(j, FREE)], pss[j][:], bias
                        )
                    else:
                        nc.scalar.activation(
                            ytile[:, bass.ts(j, FREE)],
                            pss[j][:],
                            mybir.ActivationFunctionType.Identity,
                            bias=bias,
                        )
                nc.scalar.dma_start(out=yt[n, bass.ts(m, P)], in_=ytile[:])

    nc.compile()
    return nc


def _get_program():
    if "nc" not in _prog_cache:
        _prog_cache["nc"] = _build_program()
    return _prog_cache["nc"]


def _run(x, W, b, **spmd_kwargs):
    x = np.ascontiguousarray(x, dtype=np.float32)
    W = np.ascontiguousarray(W, dtype=np.float32)
    b = np.ascontiguousarray(b, dtype=np.float32)

    # [B, n*ip] -> [n, ip, B]; two-step transpose is much faster than a
    # direct (1, 2, 0) permute copy (cache-friendly inner strides).
    xa = x.reshape(BATCH, N_COPIES, IP).transpose(1, 0, 2).astype(np.float16)
    xT = np.ascontiguousarray(xa.transpose(0, 2, 1))  # [n, ip, B] fp16
    wT = np.ascontiguousarray(W.transpose(0, 2, 1).astype(np.float16))  # [n, ip, op]

    nc = _get_program()
    in_maps = [
        {
            "xt": xT[c * BPC : (c + 1) * BPC],
            "wt": wT[c * BPC : (c + 1) * BPC],
            "bb": b[c * BPC : (c + 1) * BPC],
        }
        for c in range(N_CORES)
    ]
    res = run_bass_kernel_spmd(nc, in_maps, core_ids=list(range(N_CORES)), **spmd_kwargs)

    yT = np.concatenate([res.results[c]["yt"] for c in range(N_CORES)], axis=0).astype(np.float32)
    # [n, op, B] -> [B, n, op] -> [B, n*op]
    ya = np.ascontiguousarray(yT.transpose(0, 2, 1))  # [n, B, op]
    y = np.ascontiguousarray(ya.transpose(1, 0, 2)).reshape(BATCH, N_COPIES * OP)
    return y, res


def kernel(x, W, b):
    y, _ = _run(x, W, b)
    return y


# revision 16
# speedup vs baseline: 1.2220x; 1.1020x over previous
"""Block-diagonal linear (DiagonalLinear) Trainium2 kernel.

y[:, n*256:(n+1)*256] = x[:, n*256:(n+1)*256] @ W[n].T + b[n]  for n in 0..63

Sharding: expert-parallel over the 64 blocks — core c owns blocks
[8c, 8c+8). Host pre-transposes x and W so the contraction dim (ip) lands
on SBUF partitions; device computes yT[n, op, batch] per block with
float32r matmuls (full-rate fp32); host transposes the result back.
"""

from contextlib import ExitStack

import numpy as np

import concourse.bacc as bacc
import concourse.bass as bass
import concourse.tile as tile
from concourse import mybir
from concourse.bass_utils import run_bass_kernel_spmd

N_COPIES, IP, OP, BATCH = 64, 256, 256, 4096
N_CORES = 8
BPC = N_COPIES // N_CORES  # blocks per core
P = 128
KC = IP // P  # contraction chunks per block
MC = OP // P  # output-partition chunks per block
FREE = 512  # moving free dim per matmul (one PSUM bank of fp32)
JN = BATCH // FREE

_prog_cache = {}


def _build_program():
    nc = bacc.Bacc("TRN2", target_bir_lowering=False, debug=False)
    f32 = mybir.dt.float32
    f16 = mybir.dt.float16

    # x, W and y move as fp16: halves both input and output DMA traffic;
    # matmul accumulates in fp32 PSUM, bias-add in fp32, y rounded to fp16
    # on store (y error ~3.5e-4 scale-rel vs the fp32 reference).
    xt = nc.dram_tensor("xt", [BPC, IP, BATCH], f16, kind="ExternalInput").ap()
    # wt/bb arrive pre-packed partition-major: wt[p, n*KC+kc, o], bb[p, n*MC+m]
    wt = nc.dram_tensor("wt", [P, BPC * KC, OP], f16, kind="ExternalInput").ap()
    bb = nc.dram_tensor("bb", [P, BPC * MC], f32, kind="ExternalInput").ap()
    yt = nc.dram_tensor("yt", [BPC, OP, BATCH], f16, kind="ExternalOutput").ap()

    with tile.TileContext(nc) as tc, ExitStack() as ctx:
        const = ctx.enter_context(tc.tile_pool(name="const", bufs=1))
        xpool = ctx.enter_context(tc.tile_pool(name="x", bufs=4))
        ypool = ctx.enter_context(tc.tile_pool(name="y", bufs=6))
        psum = ctx.enter_context(tc.tile_pool(name="ps", bufs=8, space="PSUM"))

        # All weights (2 MiB) and biases for this core's 8 blocks, loaded once.
        # Input loads go on nc.sync's HWDGE ring; output stores on nc.scalar's
        # ring, so stores never head-of-line-block the next input load.
        wtile = const.tile([P, BPC * KC, OP], f16)
        nc.scalar.dma_start(out=wtile[:], in_=wt[:])
        btile = const.tile([P, BPC * MC], f32)
        nc.scalar.dma_start(out=btile[:], in_=bb[:])

        for n in range(BPC):
            xtile = xpool.tile([P, KC, BATCH], f16)
            for kc in range(KC):
                nc.sync.dma_start(out=xtile[:, kc], in_=xt[n, bass.ts(kc, P)])
            for m in range(MC):
                ytile = ypool.tile([P, BATCH], f16)
                pss = [psum.tile([P, FREE], f32, name="psj") for _ in range(JN)]
                # kc outer: the stationary weight chunk stays loaded across
                # all 8 batch chunks (1 LDWEIGHTS per 8 matmuls).
                for kc in range(KC):
                    for j in range(JN):
                        nc.tensor.matmul(
                            pss[j][:],
                            wtile[:, n * KC + kc, bass.ts(m, P)],
                            xtile[:, kc, bass.ts(j, FREE)],
                            start=(kc == 0),
                            stop=(kc == KC - 1),
                        )
                bias = btile[:, n * MC + m : n * MC + m + 1]
                for j in range(JN):
                    # split PSUM evictions across DVE and ACT
                    if j % 2 == 0:
                        nc.vector.tensor_scalar_add(
                            ytile[:, bass.ts(j, FREE)], pss[j][:], bias
                        )
                    else:
                        nc.scalar.activation(
                            ytile[:, bass.ts(j, FREE)],
                            pss[j][:],
                            mybir.ActivationFunctionType.Identity,
                            bias=bias,
                        )
                nc.scalar.dma_start(out=yt[n, bass.ts(m, P)], in_=ytile[:])

    nc.compile()
    return nc


def _get_program():
    if "nc" not in _prog_cache:
        _prog_cache["nc"] = _build_program()
    return _prog_cache["nc"]


def _prep_inputs(x, W, b):
    x = np.ascontiguousarray(x, dtype=np.float32)
    W = np.ascontiguousarray(W, dtype=np.float32)
    b = np.ascontiguousarray(b, dtype=np.float32)

    # [B, n*ip] -> [n, ip, B]; two-step transpose is much faster than a
    # direct (1, 2, 0) permute copy (cache-friendly inner strides).
    xa = x.reshape(BATCH, N_COPIES, IP).transpose(1, 0, 2).astype(np.float16)
    xT = np.ascontiguousarray(xa.transpose(0, 2, 1))  # [n, ip, B] fp16
    wT = W.transpose(0, 2, 1).astype(np.float16)  # [n, ip, op]
    # pack to [P, n*KC+kc, op]: partition p holds W rows ip = kc*P + p
    wP = np.ascontiguousarray(
        wT.reshape(N_COPIES, KC, P, OP).transpose(2, 0, 1, 3)
    )  # [P, n, KC, op]
    bP = np.ascontiguousarray(
        b.reshape(N_COPIES, MC, P).transpose(2, 0, 1)
    )  # [P, n, MC]
    return [
        {
            "xt": xT[c * BPC : (c + 1) * BPC],
            "wt": np.ascontiguousarray(
                wP[:, c * BPC : (c + 1) * BPC]
            ).reshape(P, BPC * KC, OP),
            "bb": np.ascontiguousarray(
                bP[:, c * BPC : (c + 1) * BPC]
            ).reshape(P, BPC * MC),
        }
        for c in range(N_CORES)
    ]


def _run(x, W, b, **spmd_kwargs):
    in_maps = _prep_inputs(x, W, b)
    nc = _get_program()
    res = run_bass_kernel_spmd(nc, in_maps, core_ids=list(range(N_CORES)), **spmd_kwargs)

    yT = np.concatenate([res.results[c]["yt"] for c in range(N_CORES)], axis=0).astype(np.float32)
    # [n, op, B] -> [B, n, op] -> [B, n*op]
    ya = np.ascontiguousarray(yT.transpose(0, 2, 1))  # [n, B, op]
    y = np.ascontiguousarray(ya.transpose(1, 0, 2)).reshape(BATCH, N_COPIES * OP)
    return y, res


def kernel(x, W, b):
    y, _ = _run(x, W, b)
    return y
